# revision 31
# baseline (speedup 1.0000x reference)
"""Trainium2 Bass kernel for nn_DistanceLayer (shapelet min-distance).

reference semantics:
  x: (512, 1, 2048), shapelets: (128, 1, 64)
  patches = sliding windows of x (len 64, stride 1), mean-centered
  out[b, s] = min_p ||patch(b, p) - shapelets[s]||_2          -> (512, 128)

Math (negated domain so the reduction is a MAX):
  With s~ = sh - mean_l(sh):  (w - mean(w)) . sh = w . s~
    d2[b,s,p] = A[b,p] + s2[s] - 2 w.s~,  A = sum(w^2) - (sum w)^2/64
  PE computes  v = 2 w.s~ - A  per (s, window); max_p v = s2 - min_p d2,
  so  out = sqrt(relu(s2 - max_p v)).

Layout:
  Windows p = 64j + r, j in [0,31), r in [0,64); p=1984 is the edge.
  x2T[k, (j,b)] = x[b, 64j + k], k in [0,128), built in fp16 by two
  XBAR DMA transposes of x16 (chunked [64,2048] -> [128,16,64]).
  Window (b, 64j+r) = rows [r, r+64) of column (j, b).  Two combined
  moving tiles fold the A term into the SAME K=128 contraction:
    CT1 (r<32):  rows 0..95  = x2T[0:96],   rows 96..127 = A2T[0:32]
    CT2 (r>=32): rows 32..127 = x2T[32:128], rows 0..31  = A2T[32:64]
  Weights Wn[k, r, s] = 2 s~[s, k-r] on rows [r, r+64), and -1 on the
  indicator row (96+r for r<32, r-32 for r>=32) that multiplies the A
  row, so ONE matmul per PSUM bank yields v = 2 w.s~ - A.
  A itself comes from PE ones-weight matmuls over x2T and its square
  (sliding sums), Square + subtract — no serial scan chain.
  Drain per r (PSUM [S, j, b] fp32, two 2-bank halves): r's are split
  across three engine paths chosen to balance busy time:
    'a': ACT cast to fp16 + DVE tensor_tensor max at 2x_1p
    'd': DVE tensor_reduce max over j -> strip [S, 64]
    'p': GPSIMD scalar_tensor_tensor max into an fp32 accumulator
  Final: fold accumulators/strips, v -> sqrt(relu(s2 - v)), PE
  transpose, store.

Data-parallel over 8 NeuronCores: 64 samples each, shapelets replicated.
"""

import os
import sys

import numpy as np

for _p in ("/root/.axon_site/_ro/trn_rl_repo", "/opt/trn_rl_repo"):
    if os.path.isdir(_p) and _p not in sys.path:
        sys.path.append(_p)

B, C, T = 512, 1, 2048
S, L = 128, 64
NCORES = 8
BPC = B // NCORES          # samples per core = 64
P = T - L + 1              # 1985 windows
J = 32                     # j slots (j=31 is garbage except the edge)
CB, CBO = 16, 15           # even / odd transpose chunks

# drain path per r (the PSUM-exit bottleneck: only ACT and DVE can read
# PSUM; gpsimd ALU ops, gpsimd accum-DMAs and Pool tensor ops are all
# rejected by the backend):
#   a: ACT cast -> fp16 SBUF, DVE scalar_tensor_tensor max fold
#      (TensorScalarPtr runs at 4x_2p on all-SBUF fp16 operands)
#   d: DVE tensor_reduce max over j -> strip [S, 64]
# Ratios from the engine-balance LP (a=41, d=23).
N_A = 41
PATHS = ["a" if (i + 1) * N_A // 64 > i * N_A // 64 else "d"
         for i in range(64)]

_STATE = {}

_FLAGS = {"mains": True, "drain": True}


def _build(nc, reps=1):
    import concourse.tile as tile
    from concourse import mybir

    f32 = mybir.dt.float32
    f16 = mybir.dt.float16
    OP = mybir.AluOpType
    AF = mybir.ActivationFunctionType
    AX = mybir.AxisListType.X

    x_d = nc.dram_tensor("x_shard", [BPC, T], f32, kind="ExternalInput").ap()
    wn_d = nc.dram_tensor("wn", [128, 65, S], f16, kind="ExternalInput").ap()
    wsq_d = nc.dram_tensor("wsq", [128, 65], f16, kind="ExternalInput").ap()
    wsm_d = nc.dram_tensor("wsm", [128, 65], f16, kind="ExternalInput").ap()
    nm_d = nc.dram_tensor("nmask", [128, 1], f16, kind="ExternalInput").ap()
    s2_d = nc.dram_tensor("s2v", [S, 1], f32, kind="ExternalInput").ap()
    id_d = nc.dram_tensor("ident", [128, 128], f32, kind="ExternalInput").ap()
    out_d = nc.dram_tensor("out", [BPC, S], f32, kind="ExternalOutput").ap()

    assert len(PATHS) == 64
    NS = 2 * PATHS.count("d") + 1     # two half-strips per 'd' r + edge

    with tile.TileContext(nc) as tc:
      for _it in range(reps):
        with tc.tile_pool(name=f"const{_it}", bufs=1) as constp, \
             tc.tile_pool(name=f"big{_it}", bufs=1) as bigp:

            ident = constp.tile([128, 128], f32)
            nc.scalar.dma_start(ident[:], id_d[:])
            s2 = constp.tile([S, 1], f32)
            nc.scalar.dma_start(s2[:], s2_d[:])
            nmask = constp.tile([128, 1], f16)
            nc.scalar.dma_start(nmask[:], nm_d[:])
            wsq = constp.tile([128, 65], f16)
            nc.scalar.dma_start(wsq[:], wsq_d[:])
            wsm = constp.tile([128, 65], f16)
            nc.scalar.dma_start(wsm[:], wsm_d[:])
            Wn = bigp.tile([128, 65, S], f16)
            nc.sync.dma_start(Wn[:, 0:22], wn_d[:, 0:22])
            nc.scalar.dma_start(Wn[:, 22:44], wn_d[:, 22:44])
            nc.sync.dma_start(Wn[:, 44:65], wn_d[:, 44:65])

            # x cast-loaded to fp16 (gpsimd DMAs can convert dtypes)
            x16 = bigp.tile([BPC, T], f16)
            nc.gpsimd.dma_start(x16[:], x_d[:])

            # XBAR chunk transposes: xTe[k, c, b] = x16[b, 128c + k]
            # (covers even j = 2c), xTo likewise at offset 64 (odd j).
            xTe = bigp.tile([128, CB, BPC], f16)
            nc.sync.dma_start_transpose(xTe[:], x16[:, 0:2048])
            xTo = bigp.tile([128, CBO, BPC], f16)
            nc.sync.dma_start_transpose(xTo[:], x16[:, 64:1984])

            sqe = bigp.tile([128, CB, BPC], f16)
            nc.scalar.activation(sqe[:], xTe[:], AF.Square)
            sqo = bigp.tile([128, CBO, BPC], f16)
            nc.scalar.activation(sqo[:], xTo[:], AF.Square)

            # accumulators (first touch per region is a copy, no memset):
            # DVE-owned fp16 pair + DMA-owned fp16 + strip tile
            n_a = PATHS.count("a")
            macc16_0 = (bigp.tile([S, J - 1, BPC], f16, name="macc16_0")
                        if n_a else None)
            macc16_1 = (bigp.tile([S, J - 1, BPC], f16, name="macc16_1")
                        if n_a > 1 else None)
            STR = bigp.tile([S, BPC, NS], f32)

            # CT layout [k, parity, c, b]: j = 2c + parity; (1,15) is garbage
            CT1 = bigp.tile([128, 2, CB, BPC], f16)
            CT2 = bigp.tile([128, 2, CB, BPC], f16)

            # ---- A = sum w^2 - (sum w)^2/64 via ones-weight matmuls
            with tc.tile_pool(name=f"psA{_it}", bufs=1, space="PSUM") as psA:
                eSq = psA.tile([65, CB, BPC], f32)
                eSm = psA.tile([65, CB, BPC], f32)
                oSq = psA.tile([64, CBO, BPC], f32)
                oSm = psA.tile([64, CBO, BPC], f32)
                nc.tensor.matmul(eSq[:, 0:8], wsq[:], sqe[:, 0:8],
                                 start=True, stop=True)
                nc.tensor.matmul(eSq[:, 8:16], wsq[:], sqe[:, 8:16],
                                 start=True, stop=True)
                nc.tensor.matmul(eSm[:, 0:8], wsm[:], xTe[:, 0:8],
                                 start=True, stop=True)
                nc.tensor.matmul(eSm[:, 8:16], wsm[:], xTe[:, 8:16],
                                 start=True, stop=True)
                nc.tensor.matmul(oSq[:, 0:8], wsq[:, 0:64], sqo[:, 0:8],
                                 start=True, stop=True)
                nc.tensor.matmul(oSq[:, 8:15], wsq[:, 0:64], sqo[:, 8:15],
                                 start=True, stop=True)
                nc.tensor.matmul(oSm[:, 0:8], wsm[:, 0:64], xTo[:, 0:8],
                                 start=True, stop=True)
                nc.tensor.matmul(oSm[:, 8:15], wsm[:, 0:64], xTo[:, 8:15],
                                 start=True, stop=True)

                swE = constp.tile([65, CB, BPC], f32)
                nc.scalar.activation(swE[:], eSm[:], AF.Square, scale=0.125)
                swO = constp.tile([64, CBO, BPC], f32)
                nc.scalar.activation(swO[:], oSm[:], AF.Square, scale=0.125)
                # ATe[k, c, b] = A[b, 128c + k] (k<65); ATo at offset 64
                ATe = bigp.tile([65, CB, BPC], f16)
                nc.vector.tensor_sub(ATe[:], eSq[:], swE[:])
                ATo = bigp.tile([64, CBO, BPC], f16)
                nc.vector.tensor_sub(ATo[:], oSq[:], swO[:])

            # ---- combined-tile assembly (all SBUF->SBUF DMAs on gpsimd)
            # x rows
            nc.gpsimd.dma_start(CT1[0:96, 0], xTe[0:96])
            nc.gpsimd.dma_start(CT1[0:96, 1, 0:15], xTo[0:96])
            nc.gpsimd.dma_start(CT2[32:128, 0], xTe[32:128])
            nc.gpsimd.dma_start(CT2[32:128, 1, 0:15], xTo[32:128])
            # A rows
            nc.gpsimd.dma_start(CT1[96:128, 0], ATe[0:32])
            nc.gpsimd.dma_start(CT1[96:128, 1, 0:15], ATo[0:32])
            nc.gpsimd.dma_start(CT2[0:32, 0], ATe[32:64])
            nc.gpsimd.dma_start(CT2[0:32, 1, 0:15], ATo[32:64])
            # garbage slot (1,15): zero x rows; A rows make v = -30000
            # (memsets with a partition offset are limited to 32 partitions)
            for q0 in range(0, 96, 32):
                nc.gpsimd.memset(CT1[q0:q0 + 32, 1, 15], 0.0)
                nc.gpsimd.memset(CT2[q0 + 32:q0 + 64, 1, 15], 0.0)
            nc.gpsimd.memset(CT1[96:128, 1, 15], 30000.0)
            nc.gpsimd.memset(CT2[0:32, 1, 15], 30000.0)

            # ---- edge window p=1984: rows 64..127 of the j=30 column
            with tc.tile_pool(name=f"psE{_it}", bufs=1, space="PSUM") as psE:
                pe = psE.tile([S, BPC], f32)
                nc.tensor.matmul(pe[:], Wn[:, 64, :], CT2[:, 0, 15],
                                 start=True, stop=False)
                # A[b,1984] sits at ATe[64, 15, b]; nmask is -1 at row 64
                nc.tensor.matmul(pe[:], nmask[0:65].broadcast_to([65, S]),
                                 ATe[:, 15], start=False, stop=True)
                nc.vector.tensor_copy(STR[:, :, NS - 1], pe[:])

            # ---- main sweep: one K=128 matmul per 2-bank PSUM half
            with tc.tile_pool(name=f"psM{_it}", bufs=4, space="PSUM") as psM, \
                 tc.tile_pool(name=f"drain{_it}", bufs=4) as drp:
                na = 0
                nstr = 0
                seen = set()
                for r in range(64):
                    W = Wn[:, r, :]
                    CT = CT1 if r < 32 else CT2
                    path = PATHS[r] if _FLAGS["drain"] else "n"
                    for h in range(2):
                        ph = psM.tile([S, CB, BPC], f32, tag="ph")
                        if _FLAGS["mains"]:
                            nc.tensor.matmul(ph[:, 0:8], W, CT[:, h, 0:8],
                                             start=True, stop=True)
                            nc.tensor.matmul(ph[:, 8:16], W, CT[:, h, 8:16],
                                             start=True, stop=True)
                        j0 = 16 * h
                        nv = 16 if h == 0 else 15   # drop the j=31 garbage
                        if path == "d":
                            nc.vector.tensor_reduce(
                                STR[:, :, nstr],
                                ph[:, 0:nv].rearrange("p j b -> p b j"),
                                axis=AX, op=OP.max)
                            nstr += 1
                        elif path == "a":
                            sb16 = drp.tile([S, 16, BPC], f16, tag="sb")
                            nc.scalar.mul(sb16[:, 0:nv], ph[:, 0:nv], 1.0)
                            m16 = (macc16_0, macc16_1)[na % 2]
                            if (na % 2, h) not in seen:
                                seen.add((na % 2, h))
                                nc.vector.tensor_copy(m16[:, j0:j0 + nv],
                                                      sb16[:, 0:nv])
                            else:
                                # TensorScalarPtr: 4x_2p on fp16 SBUF
                                nc.vector.scalar_tensor_tensor(
                                    m16[:, j0:j0 + nv], sb16[:, 0:nv], 1.0,
                                    m16[:, j0:j0 + nv], OP.mult, OP.max)
                    if path == "a":
                        na += 1

                # ---- finish
                r16 = constp.tile([S, BPC], f32)
                folds = [t for t in (macc16_0, macc16_1) if t is not None]
                for t in folds[1:]:
                    nc.vector.tensor_tensor(folds[0][:], folds[0][:],
                                            t[:], OP.max)
                nc.vector.tensor_reduce(
                    r16[:], folds[0][:].rearrange("p j b -> p b j"),
                    axis=AX, op=OP.max)
                rS = constp.tile([S, BPC], f32)
                nc.vector.tensor_reduce(rS[:], STR[:], axis=AX, op=OP.max)
                nc.vector.tensor_tensor(r16[:], r16[:], rS[:], OP.max)
                # d = sqrt(relu(s2 - v)):  (v - s2) clamped <= 0, Sqrt(-x)
                nc.vector.tensor_scalar(r16[:], r16[:], s2[:], 0.0,
                                        OP.subtract, OP.min)
                res = constp.tile([S, BPC], f32)
                nc.scalar.activation(res[:], r16[:], AF.Sqrt, scale=-1.0)

            with tc.tile_pool(name=f"psC{_it}", bufs=1, space="PSUM") as psC:
                po = psC.tile([BPC, S], f32)
                nc.tensor.transpose(po[:], res[:], ident[:])
                outsb = constp.tile([BPC, S], f32)
                nc.scalar.mul(outsb[:], po[:], 1.0)
                nc.sync.dma_start(out_d[:], outsb[:])


def _wn_np(sh):
    # sh: (S, L) float32 -> Wn (128, 65, S) fp16:
    #   rows [r, r+64) of slot r hold +2 s~[s, k-r]; indicator row -1.
    st = 2.0 * (sh - sh.mean(axis=1, keepdims=True))       # (S, L)
    wn = np.zeros((128, 65, S), dtype=np.float32)
    for r in range(65):
        wn[r:r + 64, r, :] = st.T
        if r < 32:
            wn[96 + r, r, :] = -1.0
        elif r < 64:
            wn[r - 32, r, :] = -1.0
    return wn.astype(np.float16)


def _wins_np():
    # ones-window weights: wsq[k, m] = 1 for k in [m, m+64)
    w = np.zeros((128, 65), dtype=np.float16)
    for m in range(65):
        w[m:m + 64, m] = 1.0
    return w


def _get_nc():
    if "nc" not in _STATE:
        from concourse import bacc
        nc = bacc.Bacc("TRN2", target_bir_lowering=False, debug=False,
                       num_devices=NCORES)
        _build(nc)
        nc.compile()
        _STATE["nc"] = nc
    return _STATE["nc"]


def _in_maps(x, shapelets):
    x = np.ascontiguousarray(np.asarray(x, dtype=np.float32)).reshape(B, T)
    sh = np.ascontiguousarray(
        np.asarray(shapelets, dtype=np.float32)).reshape(S, L)
    wn = _wn_np(sh)
    wsq = _wins_np()
    nmask = np.zeros((128, 1), dtype=np.float16)
    nmask[64, 0] = -1.0
    s2v = (sh * sh).sum(axis=1, dtype=np.float32).reshape(S, 1)
    ident = np.eye(128, dtype=np.float32)
    return [{"x_shard": x[i * BPC:(i + 1) * BPC], "wn": wn, "wsq": wsq,
             "wsm": wsq, "nmask": nmask, "s2v": s2v, "ident": ident}
            for i in range(NCORES)]


def kernel(x, shapelets):
    from concourse.bass_utils import run_bass_kernel_spmd
    nc = _get_nc()
    res = run_bass_kernel_spmd(nc, _in_maps(x, shapelets),
                               list(range(NCORES))).results
    return np.concatenate([res[i]["out"] for i in range(NCORES)], axis=0)


if __name__ == "__main__":
    rng = np.random.default_rng(0)
    x = rng.standard_normal((B, C, T)).astype(np.float32)
    sh = rng.standard_normal((S, C, L)).astype(np.float32)
    out = kernel(x, sh)
    print("out", out.shape, out.dtype, float(out.min()), float(out.max()))
